# revision 26
# baseline (speedup 1.0000x reference)
"""Trainium2 Bass kernel for nn_PrescriptionPill (segment_reduce).

Math (see reference): with xd = x (detached),
  out1[n] = x[n]@W1.T + W1_b + W2_b + loo_mean[n]@W2.T
            where loo_mean is the leave-label-out per-segment mean.
  out2[n] = LN(fc(gelu_tanh(pr)) + fc_b + pr)[seg(n)],  pr = seg_mean@proj.T + proj_b

Everything is linear in x up to the small nonlinear projection head, so:
  X1|X2|X3 = x @ [W1.T | W2.T | proj.T]        (the only big matmuls)
  out1     = X1 + b12 + A' @ X2                (A' block-diagonal per segment,
                                                scaled by 1/other_cnt, built on host
                                                from the integer labels)
  pr       = S @ X3 + proj_b                   (S = per-segment mean indicator)
then the projection head runs on [nseg, 256] only.

Sharding: data-parallel over N, aligned to label_batch segments (each of the
64 prescriptions lives on exactly one of the 8 cores); the small weights are
replicated. Segments are bin-packed into 8 cores of exactly N/8 rows via
subset-sum DP (fallback: contiguous greedy), ordered so no segment spans more
than two 128-row groups; all gathers/scatters become dense 128x128 indicator
matmuls.

The big streamed operands default to bf16 (half the DMA; PE streams 1 col/cyc
either way); the indicator/segment path runs float32r. Accumulation is fp32
PSUM throughout. KERNEL_BIG_DT=f32r switches the streams to float32r
(absmax-rel 1.8e-4 instead of 2.3e-3, ~10us slower).
"""

import numpy as np
import ml_dtypes
from contextlib import ExitStack

import concourse.bacc as bacc
import concourse.tile as tile
from concourse import mybir
from concourse.bass_utils import run_bass_kernel_spmd

F32 = mybir.dt.float32
F32R = mybir.dt.float32r
BF16 = mybir.dt.bfloat16

# dtype of the two big streamed operands (x and the stacked weights).
# bf16 halves their DMA traffic; PE streams 1 col/cycle either way.
import os
BIG_DT = {"bf16": BF16, "f32r": F32R}[os.environ.get("KERNEL_BIG_DT", "bf16")]

D = 2048
P = 256
KCH = D // 128          # 16 contraction chunks
NCORES = 8
GELU_C0 = 0.7978845608028654
GELU_C1 = 0.044715
LN_EPS = 1e-5

_PROG_CACHE = {}


# ----------------------------------------------------------------------------
# host-side planning
# ----------------------------------------------------------------------------

def _pack_bin(idx_list, seg_cnts, seg_starts):
    """Pack the given segments in order; returns (seg_meta, slots_used)."""
    q = 0
    seg_meta = []
    for s in idx_list:
        cnt = int(seg_cnts[s])
        assert cnt <= 256, "segment larger than two row groups"
        if cnt > 128 and (q % 128) + cnt > 256:
            q = ((q + 127) // 128) * 128
        seg_meta.append((q, cnt, int(seg_starts[s])))
        q += cnt
    return seg_meta, q


def _order_bin(items, seg_cnts):
    """Order a bin's segments so no segment spans three 128-row groups,
    assuming zero padding (exact packing). None if impossible."""
    import itertools
    szs = [int(seg_cnts[i]) for i in items]
    if len(items) > 9:
        return None
    for perm in itertools.permutations(range(len(items))):
        p = 0
        ok = True
        for j in perm:
            s = szs[j]
            if s > 128 and (p % 128) + s > 256:
                ok = False
                break
            p += s
        if ok:
            return [items[j] for j in perm]
    return None


def _exact_partition(sizes, nbins, target, tries=64, seed=0):
    """Partition indices into nbins bins each summing exactly to target
    (subset-sum DP, randomized restarts). None if not found."""
    import random
    rng = random.Random(seed)
    n = len(sizes)
    for _ in range(tries):
        order = list(range(n))
        rng.shuffle(order)
        remaining = set(order)
        bins = []
        ok = True
        for _b in range(nbins - 1):
            items = [i for i in order if i in remaining]
            parent = {0: None}
            for i in items:
                s_i = int(sizes[i])
                new = {}
                for s in list(parent.keys()):
                    ns = s + s_i
                    if ns <= target and ns not in parent and ns not in new:
                        new[ns] = (s, i)
                parent.update(new)
                if target in parent:
                    break
            if target not in parent:
                ok = False
                break
            sel = []
            s = target
            while parent[s] is not None:
                ps, i = parent[s]
                sel.append(i)
                s = ps
            bins.append(sel)
            remaining -= set(sel)
        if ok:
            last = list(remaining)
            if sum(int(sizes[i]) for i in last) == target:
                bins.append(last)
                return bins
    return None


def _plan(label, label_batch):
    """Shard segments across cores. Preferred: an exact 8-way partition with
    every core at exactly N/8 rows (minimal row groups, zero padding).
    Fallback: contiguous ranges balanced greedily."""
    N = label_batch.shape[0]
    segs, seg_starts, seg_cnts = np.unique(label_batch, return_index=True,
                                           return_counts=True)
    nseg = len(segs)

    if N % NCORES == 0 and N // NCORES % 128 == 0:
        target = N // NCORES
        bins = _exact_partition(seg_cnts, NCORES, target)
        if bins is not None:
            ordered = [_order_bin(b, seg_cnts) for b in bins]
            if all(o is not None for o in ordered):
                cores = []
                ok = True
                for o in ordered:
                    seg_meta, q = _pack_bin(o, seg_cnts, seg_starts)
                    if q != target:  # padding crept in: ordering failed
                        ok = False
                        break
                    cores.append(seg_meta)
                if ok:
                    NG = target // 128
                    return cores, NG, max(len(b) for b in bins)

    # fallback: contiguous, greedy-balanced
    cum = np.cumsum(seg_cnts)
    bounds = [0]
    for c in range(1, NCORES):
        t = N * c / NCORES
        i = int(np.argmin(np.abs(cum - t))) + 1
        bounds.append(max(i, bounds[-1] + 1))
    bounds.append(nseg)

    cores = []
    maxslots = 0
    maxseg = 0
    for c in range(NCORES):
        s0, s1 = bounds[c], bounds[c + 1]
        seg_meta, q = _pack_bin(list(range(s0, s1)), seg_cnts, seg_starts)
        cores.append(seg_meta)
        maxslots = max(maxslots, q)
        maxseg = max(maxseg, s1 - s0)

    NG = (maxslots + 127) // 128
    return cores, NG, maxseg


def _pack_core(seg_meta, x, label, NG, NSEG, pairs, pair_map):
    """Build this core's device input tensors."""
    NMAX = NG * 128
    NPAIR = len(pairs)
    slots_list, rows_list = [], []
    for (q0, cnt, r0) in seg_meta:
        slots_list.append(np.arange(q0, q0 + cnt))
        rows_list.append(np.arange(r0, r0 + cnt))
    slots = np.concatenate(slots_list)
    rows = np.concatenate(rows_list)

    xp = np.zeros((NMAX, D), np.float32)
    xp[slots] = x[rows]
    # xTg[g, p, k*128+n] = xp[g*128+n, k*128+p]
    xTg = np.ascontiguousarray(
        xp.reshape(NG, 128, KCH, 128).transpose(0, 3, 2, 1))

    a3T = np.zeros((NPAIR, 128, 128), np.float32)
    segind = np.zeros((128, NG, NSEG), np.float32)
    for ls, (q0, cnt, r0) in enumerate(seg_meta):
        l = label[r0:r0 + cnt]
        same = l[:, None] == l[None, :]
        other_cnt = cnt - same.sum(1)
        coef = np.where(other_cnt > 0, 1.0 / np.maximum(other_cnt, 1), 0.0)
        M = (~same) * coef[None, :].astype(np.float32)
        si = np.arange(q0, q0 + cnt)
        pidx = pair_map[si[:, None] // 128, si[None, :] // 128]
        assert (pidx >= 0).all()
        flat = (pidx * 128 + (si % 128)[:, None]) * 128 + (si % 128)[None, :]
        a3T.reshape(-1)[flat.ravel()] = M.ravel().astype(np.float32)
        segind[si % 128, si // 128, ls] = 1.0 / cnt

    # a3T device layout: [src_r, pair, tgt_r]
    a3T_dev = np.ascontiguousarray(a3T.transpose(1, 0, 2))
    return xTg, a3T_dev, segind, slots, rows


# ----------------------------------------------------------------------------
# device program
# ----------------------------------------------------------------------------

def _build_program(NG, NSEG, NPAIR, pairs, ln_identity=False):
    nc = bacc.Bacc("TRN2", target_bir_lowering=False, debug=False)
    NMAX = NG * 128

    xTg = nc.dram_tensor("xTg", [NG, 128, KCH * 128], BIG_DT, kind="ExternalInput").ap()
    wT = nc.dram_tensor("wT", [KCH, 128, 3 * P], BIG_DT, kind="ExternalInput").ap()
    a3T = nc.dram_tensor("a3T", [128, NPAIR, 128], BIG_DT, kind="ExternalInput").ap()
    segind = nc.dram_tensor("segind", [128, NG, NSEG], F32R, kind="ExternalInput").ap()
    b12 = nc.dram_tensor("b12", [128, P], F32, kind="ExternalInput").ap()
    fcT = nc.dram_tensor("fcT", [128, 2, P], BIG_DT, kind="ExternalInput").ap()
    projb = nc.dram_tensor("projb", [NSEG, P], F32, kind="ExternalInput").ap()
    fcb = nc.dram_tensor("fcb", [NSEG, P], F32, kind="ExternalInput").ap()
    lng = nc.dram_tensor("lng", [NSEG, P], F32, kind="ExternalInput").ap()
    lnb = nc.dram_tensor("lnb", [NSEG, P], F32, kind="ExternalInput").ap()
    ident = nc.dram_tensor("ident", [NSEG, NSEG], F32, kind="ExternalInput").ap()
    y1 = nc.dram_tensor("y1", [NMAX, P], F32, kind="ExternalOutput").ap()
    y2 = nc.dram_tensor("y2", [NSEG, P], F32, kind="ExternalOutput").ap()

    # pairs with a given target group, as (src_group, pair_index)
    tgt_pairs = {t: [] for t in range(NG)}
    for pi, (sg, tg) in enumerate(pairs):
        tgt_pairs[tg].append((sg, pi))

    with tile.TileContext(nc) as tc:
        with ExitStack() as ctx:
            big = ctx.enter_context(tc.tile_pool(name="big", bufs=1))
            x3p = ctx.enter_context(tc.tile_pool(name="x3p", bufs=3))
            y1p = ctx.enter_context(tc.tile_pool(name="y1p", bufs=3))
            tail = ctx.enter_context(tc.tile_pool(name="tail", bufs=1))
            pA = ctx.enter_context(tc.tile_pool(name="pA", bufs=2, space="PSUM"))
            pB = ctx.enter_context(tc.tile_pool(name="pB", bufs=2, space="PSUM"))
            pS = ctx.enter_context(tc.tile_pool(name="pS", bufs=1, space="PSUM"))
            pT = ctx.enter_context(tc.tile_pool(name="pT", bufs=1, space="PSUM"))

            # ---- input loads ----
            # Order matters: the DMA stream is ~bandwidth-serial, and PE's
            # first group needs xg0 + W chunks in k order. Everything later
            # (xg1.., a3, tail constants) follows.
            xg_sb = big.tile([128, NG, KCH * 128], BIG_DT)
            w_sb = big.tile([128, KCH, 3 * P], BIG_DT)
            Q = KCH * 128 // 4

            def xq(g, q):
                nc.sync.dma_start(out=xg_sb[:, g, q * Q:(q + 1) * Q],
                                  in_=xTg[g][:, q * Q:(q + 1) * Q])

            # xg0 quarters ride half a W-block ahead of the chunks that need
            # them; xg1 quarters slot into the late W stream so group 1 can
            # start the moment group 0's matmuls finish.
            xq(0, 0)
            for k in range(KCH):
                if k == 2:
                    xq(0, 1)
                elif k == 6:
                    xq(0, 2)
                elif k == 10:
                    xq(0, 3)
                elif k == 12:
                    xq(1, 0)
                elif k == 14:
                    xq(1, 1)
                nc.sync.dma_start(out=w_sb[:, k, :], in_=wT[k])
            xq(1, 2)
            xq(1, 3)
            nc.sync.dma_start(out=xg_sb[:, 2, :], in_=xTg[2])
            si_sb = big.tile([128, NG, NSEG], F32R)
            nc.sync.dma_start(out=si_sb, in_=segind)
            b12_sb = big.tile([128, P], F32)
            nc.sync.dma_start(out=b12_sb, in_=b12)
            fcT_sb = big.tile([128, 2, P], BIG_DT)
            nc.sync.dma_start(out=fcT_sb, in_=fcT)
            projb_sb = big.tile([NSEG, P], F32)
            nc.sync.dma_start(out=projb_sb, in_=projb)
            fcb_sb = big.tile([NSEG, P], F32)
            nc.sync.dma_start(out=fcb_sb, in_=fcb)
            lng_sb = big.tile([NSEG, P], F32)
            nc.sync.dma_start(out=lng_sb, in_=lng)
            lnb_sb = big.tile([NSEG, P], F32)
            nc.sync.dma_start(out=lnb_sb, in_=lnb)
            id_sb = big.tile([NSEG, NSEG], F32)
            nc.sync.dma_start(out=id_sb, in_=ident)
            a3_sb = big.tile([128, NPAIR, 128], BIG_DT)
            nc.sync.dma_start(out=a3_sb, in_=a3T)
            for g in range(3, NG):
                nc.sync.dma_start(out=xg_sb[:, g, :], in_=xTg[g])

            x1b_sb = big.tile([128, NG, P], F32)
            x2_sb = big.tile([128, NG, P], BIG_DT)
            psS = pS.tile([NSEG, P], F32)

            # Warm the ACT function tables (Tanh/Copy/Sqrt) during the DMA
            # phase so LoadActFuncSet is off the critical path of the tail.
            warm = tail.tile([1, 1], F32, tag="warm")
            nc.vector.memset(warm, 0.0)
            warm2 = tail.tile([1, 1], F32, tag="warm2")
            nc.scalar.activation(warm2, warm, mybir.ActivationFunctionType.Sqrt,
                                 bias=warm)
            nc.scalar.activation(warm2, warm,
                                 mybir.ActivationFunctionType.Gelu_apprx_tanh)

            def emit_b(t):
                plist = tgt_pairs[t]
                psB = pB.tile([128, P], F32, tag="pB")
                for i, (sg, pi) in enumerate(plist):
                    nc.tensor.matmul(psB, a3_sb[:, pi, :], x2_sb[:, sg, :],
                                     start=(i == 0), stop=(i == len(plist) - 1))
                y1t = y1p.tile([128, P], F32, tag="y1t")
                nc.vector.tensor_add(y1t, x1b_sb[:, t, :], psB)
                nc.sync.dma_start(out=y1[t * 128:(t + 1) * 128, :], in_=y1t)

            # ---- main loop: X123 matmuls + per-group epilogues ----
            def emit_a_mms(g, psA, k):
                lhsT = xg_sb[:, g, k * 128:(k + 1) * 128]
                nc.tensor.matmul(psA[:, 0:512], lhsT, w_sb[:, k, 0:512],
                                 start=(k == 0), stop=(k == KCH - 1))
                nc.tensor.matmul(psA[:, 512:768], lhsT, w_sb[:, k, 512:768],
                                 start=(k == 0), stop=(k == KCH - 1))

            x3_tiles = {}

            def emit_psS(g):
                nc.tensor.matmul(psS, si_sb[:, g, :], x3_tiles.pop(g),
                                 start=(g == 0), stop=(g == NG - 1))

            def emit_epilogue(g, psA):
                nc.vector.tensor_add(x1b_sb[:, g, :], b12_sb, psA[:, 0:P])
                nc.vector.tensor_copy(x2_sb[:, g, :], psA[:, P:2 * P])
                x3t = x3p.tile([128, P], F32R, tag="x3t")
                nc.vector.tensor_copy(x3t, psA[:, 2 * P:3 * P])
                x3_tiles[g] = x3t
                # defer the segment-sum matmul of the previous group so it
                # never stalls the in-order PE queue on this group's DVE chain
                if g >= 1:
                    emit_psS(g - 1)

            for g in range(NG):
                psA = pA.tile([128, 3 * P], F32, tag="pA")
                for k in range(KCH):
                    emit_a_mms(g, psA, k)
                emit_epilogue(g, psA)
                if g >= 2:
                    emit_b(g - 2)

            emit_psS(NG - 1)

            # ---- projection head on [NSEG, 256] ----
            # DVE/ACT part overlaps with the remaining A'/output matmuls.
            pr = tail.tile([NSEG, P], F32, tag="pr")
            nc.vector.tensor_add(pr, projb_sb, psS)

            emit_b(NG - 2)

            gT = tail.tile([128, 2, NSEG], BIG_DT, tag="gT")
            for c in range(2):
                ptr = pT.tile([128, NSEG], F32, tag="ptr")
                nc.tensor.transpose(ptr, pr[:, c * 128:(c + 1) * 128], id_sb)
                nc.scalar.activation(gT[:, c, :], ptr,
                                     mybir.ActivationFunctionType.Gelu_apprx_tanh)

            emit_b(NG - 1)

            prb = tail.tile([NSEG, P], F32, tag="prb")
            nc.vector.tensor_add(prb, pr, fcb_sb)
            psF = pB.tile([128, P], F32, tag="pB")
            for c in range(2):
                nc.tensor.matmul(psF[0:NSEG, :], gT[:, c, :], fcT_sb[:, c, :],
                                 start=(c == 0), stop=(c == 1))
            t_h = tail.tile([NSEG, P], F32, tag="t_h")
            nc.vector.tensor_add(t_h, prb, psF[0:NSEG, :])

            stats = tail.tile([NSEG, 6], F32, tag="stats")
            nc.vector.bn_stats(out=stats, in_=t_h)
            mv = tail.tile([NSEG, 2], F32, tag="mv")
            nc.vector.bn_aggr(out=mv, in_=stats)
            epst = tail.tile([NSEG, 1], F32, tag="epst")
            nc.vector.memset(epst, LN_EPS)
            sd = tail.tile([NSEG, 1], F32, tag="sd")
            nc.scalar.activation(sd, mv[:, 1:2], mybir.ActivationFunctionType.Sqrt,
                                 bias=epst)
            rstd = tail.tile([NSEG, 1], F32, tag="rstd")
            nc.vector.reciprocal(rstd, sd)
            t_dn = tail.tile([NSEG, P], F32, tag="t_dn")
            nc.vector.tensor_scalar(t_dn, t_h, mv[:, 0:1], rstd,
                                    op0=mybir.AluOpType.subtract,
                                    op1=mybir.AluOpType.mult)
            if ln_identity:
                # ln_g == 1, ln_b == 0 (checked on host): affine is a no-op
                nc.sync.dma_start(out=y2, in_=t_dn)
            else:
                t_y2g = tail.tile([NSEG, P], F32, tag="t_y2g")
                nc.vector.tensor_mul(t_y2g, t_dn, lng_sb)
                t_y2 = tail.tile([NSEG, P], F32, tag="t_y2")
                nc.vector.tensor_add(t_y2, t_y2g, lnb_sb)
                nc.sync.dma_start(out=y2, in_=t_y2)

    nc.compile()
    return nc


# ----------------------------------------------------------------------------
# entry point
# ----------------------------------------------------------------------------

def kernel(x, label, label_batch, W1_w, W1_b, W2_w, W2_b,
           proj_w, proj_b, fc_w, fc_b, ln_g, ln_b):
    x = np.asarray(x, np.float32)
    label = np.asarray(label)
    label_batch = np.asarray(label_batch)
    N = x.shape[0]

    cores, NG, NSEG = _plan(label, label_batch)
    pairs = ([(g, g) for g in range(NG)]
             + [(g, g + 1) for g in range(NG - 1)]
             + [(g + 1, g) for g in range(NG - 1)])
    NPAIR = len(pairs)
    pair_map = -np.ones((NG, NG), np.int64)
    for pi, (sg, tg) in enumerate(pairs):
        pair_map[sg, tg] = pi

    ln_identity = bool(np.all(np.asarray(ln_g) == 1.0)
                       and np.all(np.asarray(ln_b) == 0.0))
    key = (NG, NSEG, NPAIR, ln_identity)
    if key not in _PROG_CACHE:
        _PROG_CACHE[key] = _build_program(NG, NSEG, NPAIR, pairs,
                                          ln_identity=ln_identity)
    nc = _PROG_CACHE[key]

    # replicated weights
    W123T = np.ascontiguousarray(
        np.concatenate([np.asarray(W1_w).T, np.asarray(W2_w).T,
                        np.asarray(proj_w).T], axis=1).astype(np.float32))
    wT_dev = np.ascontiguousarray(W123T.reshape(KCH, 128, 3 * P))
    if BIG_DT == BF16:
        wT_dev = wT_dev.astype(ml_dtypes.bfloat16)
    b12_dev = np.ascontiguousarray(
        np.broadcast_to((np.asarray(W1_b) + np.asarray(W2_b)).astype(np.float32),
                        (128, P)))
    fcT_dev = np.ascontiguousarray(
        np.asarray(fc_w).T.astype(np.float32).reshape(2, 128, P).transpose(1, 0, 2))
    if BIG_DT == BF16:
        fcT_dev = fcT_dev.astype(ml_dtypes.bfloat16)

    def rep(v):
        return np.ascontiguousarray(
            np.broadcast_to(np.asarray(v).astype(np.float32), (NSEG, P)))

    projb_dev, fcb_dev = rep(proj_b), rep(fc_b)
    lng_dev, lnb_dev = rep(ln_g), rep(ln_b)
    ident_dev = np.eye(NSEG, dtype=np.float32)

    in_maps = []
    packs = []
    for c in range(NCORES):
        xTg, a3T_dev, segind_dev, slots, rows = _pack_core(
            cores[c], x, label, NG, NSEG, pairs, pair_map)
        if BIG_DT == BF16:
            xTg = xTg.astype(ml_dtypes.bfloat16)
            a3T_dev = a3T_dev.astype(ml_dtypes.bfloat16)
        packs.append((slots, rows, cores[c]))
        in_maps.append({
            "xTg": xTg, "wT": wT_dev, "a3T": a3T_dev, "segind": segind_dev,
            "b12": b12_dev, "fcT": fcT_dev, "projb": projb_dev, "fcb": fcb_dev,
            "lng": lng_dev, "lnb": lnb_dev, "ident": ident_dev,
        })

    res = run_bass_kernel_spmd(nc, in_maps, list(range(NCORES)))

    out1 = np.zeros((N, P), np.float32)
    out2 = np.zeros((N, P), np.float32)
    for c in range(NCORES):
        slots, rows, seg_meta = packs[c]
        out1[rows] = res.results[c]["y1"][slots]
        y2c = res.results[c]["y2"]
        for ls, (q0, cnt, r0) in enumerate(seg_meta):
            out2[r0:r0 + cnt] = y2c[ls]
    return out1, out2
